# revision 25
# baseline (speedup 1.0000x reference)
"""Trainium2 Bass kernel for nn_NeighborhoodAttentionModule.

Pure data-parallel over batch: B=16384 rows split as 2048 rows/core across 8
NeuronCores, 16 b-tiles of 128 rows per core. Per b-tile:

  s1T[b,(h,a)]   = ceT8.T @ U2         (PE DoubleRow fp8, 1 matmul)
  z[(h,a),(b,k)] = VU.T @ neT8 + s1T-broadcast   (PE: fp8 DoubleRow + fp16
                   rank-expand matmul accumulated into same PSUM)
  h = tanh(z)                          (ACT, PSUM->SBUF fp16)
  raw[(b,k),(c,h)] = h_chunk.T @ w2    (PE chunk-stationary, 16 tiny matmuls)
  rawn = raw + nwv                     (DVE; nwv = valid ? nw : -30, host-folded)
  em = exp(rawn)                       (ACT fp16; invalid -> exp(-30+raw) == 0)
  S[(b',h),(c,h)] via bm8 matmul; recS = 1/(S+2e-5)  (PE + DVE)
  recSmap = bm8T @ recS                (PE partition-broadcast)
  p = em * recSmap; expblk = p * blockmask           (DVE, fp16)
  aggT[dd,(b,h)] += ner8_chunk.T @ expblk_chunk      (PE, fp8 x fp16 mixed)
  fused = aggT.T @ Wcc + bc            (PE)
  out = LayerNorm(fused + center)      (DVE only: STT-accum stats + int
                                        rsqrt bithack + 2 Newton steps)

Zero ACT table swaps (Tanh+Exp share the exp_and_others table). All DRAM
tensors are laid out host-side as per-tile SBUF images (4KB/512B contiguous
partition lines) for dense DMA descriptors.
"""
import os
import numpy as np

B, K, D, H, A = 16384, 16, 256, 2, 64
TBLOB = 8736          # per-tile input blob bytes per partition
CBLOB = 1946          # fp16 const blob columns
NCORES = 8
BC = B // NCORES      # rows per core (2048)
NBT = BC // 128       # b-tiles per core (16)
NCH = 16              # chunks of 128 (b,k)-rows per b-tile
HA = H * A            # 128
EPS = 1e-5
NWV_NEG = -30.0       # folded invalid-neighbor bias
S_EPS = 2e-5          # S regularizer (valid rows always have S >= 1.5e-3)
RSQRT_MAGIC = 0x5F3759DF

LAST_EXEC_NS = None

_prog_cache = {}


def _maybe_install_profile_hook():
    """Optional NTFF profiling hook (for local testing only; fails soft)."""
    import sys, types, contextlib, ctypes

    if "antenv.axon_hooks" in sys.modules:
        return
    try:
        mod = types.ModuleType("antenv.axon_hooks")
        _state = {"hook": None}
        mod.set_axon_ntff_profile_hook = lambda h: _state.__setitem__("hook", h)
        mod.get_axon_ntff_profile_hook = lambda: _state["hook"]
        sys.modules["antenv.axon_hooks"] = mod
        import antenv

        antenv.axon_hooks = mod
        so_path = "/opt/axon/libaxon_pjrt.so"
        lib = ctypes.CDLL(so_path)
        if not hasattr(lib, "axon_start_nrt_profile"):
            return
        lib.axon_start_nrt_profile.argtypes = [
            ctypes.POINTER(ctypes.c_int64),
            ctypes.c_size_t,
        ]
        lib.axon_start_nrt_profile.restype = ctypes.c_int64
        lib.axon_stop_nrt_profile.argtypes = [ctypes.c_char_p]
        lib.axon_stop_nrt_profile.restype = ctypes.c_int64

        @contextlib.contextmanager
        def _hook(output_dir, device_ids):
            import jax

            jax.devices()
            if device_ids:
                ids = (ctypes.c_int64 * len(device_ids))(*device_ids)
                rc = lib.axon_start_nrt_profile(ids, len(device_ids))
            else:
                rc = lib.axon_start_nrt_profile(None, 0)
            if rc != 0:
                raise RuntimeError(f"axon_start_nrt_profile rc={rc}")
            try:
                yield
            finally:
                n = lib.axon_stop_nrt_profile(str(output_dir).encode())
                print(f"profile: {n} ntff file(s) -> {output_dir}")

        mod.set_axon_ntff_profile_hook(_hook)
    except Exception as e:  # noqa: BLE001
        print("profile hook unavailable:", e)


def _build_program(apply_gamma_beta: bool, apply_b1: bool):
    from concourse import bacc, tile, mybir

    F8 = mybir.dt.float8e4
    F16 = mybir.dt.float16
    F32 = mybir.dt.float32
    I32 = mybir.dt.int32
    AFT = mybir.ActivationFunctionType
    ALU = mybir.AluOpType
    PM = mybir.MatmulPerfMode

    nc = bacc.Bacc(None, target_bir_lowering=False)

    # ---- DRAM parameters (per-core shard; all per-tile SBUF images) ----
    dp = nc.declare_dram_parameter
    # per-tile blob: neT8 4096B | ner8 4096B | ce16 512B | nwv16 32B
    blob_d = dp("blob8", [NBT, 128, TBLOB], F8, isOutput=False)
    ceT_d = dp("ceT8", [128, 2, BC], F8, isOutput=False)
    vu2_d = dp("vu2_8", [128, 2, 2 * HA], F8, isOutput=False)  # vu | u2
    # fp16 const blob: w2p 2 | bm8 8 | bm16 16 | wcc 1024 | bm8T 128 |
    # idk 512 | ones 128 | b1 128
    cb_d = dp("cblob16", [128, CBLOB], F16, isOutput=False)
    gam_d = dp("gamma_r", [1, D], F32, isOutput=False)
    bet_d = dp("beta_r", [1, D], F32, isOutput=False)
    out_d = dp("out", [BC, D], F32, isOutput=True)
    debug = bool(os.environ.get("NE_DEBUG_DUMP"))
    if debug:
        dbg_h_d = dp("dbg_h", [128, 2048], F16, isOutput=True)
        dbg_rawn_d = dp("dbg_rawn", [128, NCH, H], F32, isOutput=True)
        dbg_em_d = dp("dbg_em", [128, NCH, H], F16, isOutput=True)
        dbg_p_d = dp("dbg_p", [128, NCH, H], F16, isOutput=True)
        dbg_aggT_d = dp("dbg_aggT", [128, 2, 2 * 128], F16, isOutput=True)
        dbg_x_d = dp("dbg_x", [128, D], F32, isOutput=True)

    with tile.TileContext(nc) as tc:
        with (
            tc.tile_pool(name="const", bufs=1) as cpool,
            tc.tile_pool(name="loads", bufs=5) as lpool,
            tc.tile_pool(name="work", bufs=2) as wpool,
            tc.tile_pool(name="zps", bufs=2, space="PSUM") as zps_p,
            tc.tile_pool(name="srm_ps", bufs=1, space="PSUM") as srm_p,
            tc.tile_pool(name="aggps", bufs=1, space="PSUM") as aggps_p,
            tc.tile_pool(name="fups", bufs=1, space="PSUM") as fups_p,
        ):
            def cload(name, dram_ap, shape, dt):
                t = cpool.tile(shape, dt, tag=name, name=name)
                nc.sync.dma_start(t[:], dram_ap)
                return t

            ceT8 = cload("ceT8", ceT_d[:], [128, 2, BC], F8)
            vu2 = cload("vu2", vu2_d[:], [128, 2, 2 * HA], F8)
            vu8 = vu2[:, :, 0:HA]
            u28 = vu2[:, :, HA:2 * HA]
            cb = cload("cb", cb_d[:], [128, CBLOB], F16)
            w2p = cb[:, 0:2]
            bm8 = cb[:, 2:10]
            bm16 = cb[:, 10:26].rearrange("p (b h) -> p b h", h=H)
            wcc = [[cb[:, 26 + (h * 2 + dh) * D:26 + (h * 2 + dh + 1) * D]
                    for dh in range(2)] for h in range(2)]
            bm8T = cb[0:8, 1050:1178]
            if apply_b1:
                b1c = cpool.tile([128, 1], F32, tag="b1c")
                nc.vector.tensor_copy(b1c[:], cb[:, 1818:1819])
            gam_t = (
                cload("gam", gam_d[:].to_broadcast((128, D)), [128, D], F32)
                if apply_gamma_beta else None
            )
            bet_t = (
                cload("bet", bet_d[:].to_broadcast((128, D)), [128, D], F32)
                if apply_gamma_beta else None
            )

            def issue_loads(t):
                blob = lpool.tile([128, TBLOB], F8, tag="blob")
                nc.sync.dma_start(blob[:], blob_d[t])
                neT = blob[:, 0:4096].rearrange("p (i c) -> p i c", i=2)
                ner = blob[:, 4096:8192].rearrange("p (c d) -> p c d", c=NCH)
                cen = blob[:, 8192:8704].bitcast(F16)
                nwv = blob[:, 8704:8736].bitcast(F16)
                return neT, ner, cen, nwv

            def issue_fused(t, aggT, cen):
                # fused = combined @ Wc (bc folded into cen host-side)
                fu_ps = fups_p.tile([128, D], F32, tag="fu", bufs=2)
                mms = [(h, dh) for h in range(2) for dh in range(2)]
                for i, (h, dh) in enumerate(mms):
                    lhs = aggT[:, dh].rearrange("p (b h) -> p h b", h=2)[:, h, :]
                    nc.tensor.matmul(
                        fu_ps[:], lhs, wcc[h][dh],
                        start=(i == 0), stop=(i == 3),
                    )
                return (t, fu_ps, cen)

            def issue_ln(t, fu_ps, cen):
                # residual + layernorm (all DVE)
                x_t = wpool.tile([128, D], F32, tag="x")
                msum = wpool.tile([128, 1], F32, tag="msum")
                nc.vector.scalar_tensor_tensor(
                    x_t[:], fu_ps[:], 1.0, cen[:],
                    op0=ALU.mult, op1=ALU.add, accum_out=msum[:],
                )
                if debug and t == 0:
                    nc.sync.dma_start(dbg_x_d[:], x_t[:])
                negmean = wpool.tile([128, 1], F32, tag="negmean")
                nc.vector.tensor_scalar_mul(negmean[:], msum[:], -1.0 / D)
                sq_t = wpool.tile([128, D], F32, tag="sq")
                sumsq = wpool.tile([128, 1], F32, tag="sumsq")
                nc.vector.scalar_tensor_tensor(
                    sq_t[:], x_t[:], 1.0, x_t[:],
                    op0=ALU.mult, op1=ALU.mult, accum_out=sumsq[:],
                )
                m2 = wpool.tile([128, 1], F32, tag="m2")
                nc.vector.tensor_mul(m2[:], negmean[:], negmean[:])
                q_t = wpool.tile([128, 1], F32, tag="q")
                nc.vector.tensor_scalar(
                    q_t[:], sumsq[:], 1.0 / D, EPS, op0=ALU.mult, op1=ALU.add,
                )
                nc.vector.tensor_sub(q_t[:], q_t[:], m2[:])
                # invstd = rsqrt(q): int bithack + 2 Newton steps
                yi = wpool.tile([128, 1], I32, tag="yi")
                nc.vector.tensor_scalar(
                    yi[:], q_t[:].bitcast(I32), 1, None,
                    op0=ALU.logical_shift_right,
                )
                nc.vector.tensor_scalar(
                    yi[:], yi[:], RSQRT_MAGIC, -1,
                    op0=ALU.subtract, op1=ALU.mult,
                )
                y = yi[:].bitcast(F32)
                nr1 = wpool.tile([128, 1], F32, tag="nr1")
                nr2 = wpool.tile([128, 1], F32, tag="nr2")
                for _ in range(2):
                    nc.vector.tensor_mul(nr1[:], y, y)
                    nc.vector.scalar_tensor_tensor(
                        nr2[:], q_t[:], -0.5, nr1[:], op0=ALU.mult, op1=ALU.mult,
                    )
                    nc.vector.tensor_scalar(nr1[:], nr2[:], 1.5, None, op0=ALU.add)
                    nc.vector.tensor_mul(yi[:].bitcast(F32), y, nr1[:])
                xn = wpool.tile([128, D], F32, tag="xn")
                nc.vector.tensor_scalar(
                    xn[:], x_t[:], negmean[:], yi[:].bitcast(F32),
                    op0=ALU.add, op1=ALU.mult,
                )
                if apply_gamma_beta:
                    nc.vector.tensor_mul(xn[:], xn[:], gam_t[:])
                    nc.vector.tensor_add(xn[:], xn[:], bet_t[:])
                nc.sync.dma_start(out_d[t * 128:(t + 1) * 128, :], xn[:])

            def issue_z_half(t, neT, h_sb, hf):
                # z = VU.T @ neT8 + U2.T @ ceT8-kexp (all fp8 DoubleRow; the
                # ce pass uses a stride-0 k-broadcast moving AP).
                z_ps = zps_p.tile([128, 1024], F32, tag="z")
                for m in range(4):
                    c0 = hf * 1024 + m * 256
                    b0 = t * 128 + hf * 64 + m * 16
                    zq = z_ps[:, m * 256:(m + 1) * 256]
                    nc.tensor.matmul(
                        zq, vu8, neT[:, :, c0:c0 + 256],
                        start=True, stop=False, perf_mode=PM.DoubleRow,
                        skip_group_check=True,
                    )
                    ce_kexp = ceT8[:, :, b0:b0 + 16][:, :, :, None] \
                        .to_broadcast((128, 2, 16, 16))
                    nc.tensor.matmul(
                        zq, u28, ce_kexp,
                        start=False, stop=True, perf_mode=PM.DoubleRow,
                        skip_group_check=True,
                    )
                return z_ps

            def issue_tanh(h_sb, z_ps, hf):
                if apply_b1:
                    nc.scalar.activation(
                        h_sb[:, hf * 1024:(hf + 1) * 1024], z_ps[:],
                        AFT.Tanh, bias=b1c[:],
                    )
                else:
                    nc.scalar.activation(
                        h_sb[:, hf * 1024:(hf + 1) * 1024], z_ps[:], AFT.Tanh,
                    )

            # ---- software-pipelined main loop ----
            # iteration t: raw/em/S/agg for tile t, fused+LN for t-1,
            # z+tanh for t+1, blob prefetch for t+2.
            tiles = {0: issue_loads(0)}
            tiles[1] = issue_loads(1)
            h_cur = wpool.tile([128, 2048], F16, tag="h")
            for hf in range(2):
                zp = issue_z_half(0, tiles[0][0], h_cur, hf)
                issue_tanh(h_cur, zp, hf)
            pending = []
            for t in range(NBT):
                neT, ner, cen, nwv = tiles[t]
                if t + 2 < NBT:
                    tiles[t + 2] = issue_loads(t + 2)
                if debug and t == 0:
                    nc.sync.dma_start(dbg_h_d[:], h_cur[:])

                # ---- raw scores (chunk-stationary) + nwv add ----
                srm = srm_p.tile([128, NCH, H], F32, tag="srm")
                raw_ps = srm
                for c in range(NCH):
                    nc.tensor.matmul(
                        raw_ps[:, c, :],
                        h_cur[:, c * 128:(c + 1) * 128], w2p,
                        start=True, stop=True,
                    )
                rawn = wpool.tile([128, NCH, H], F32, tag="rawn")
                nc.vector.tensor_add(
                    rawn[:], raw_ps[:],
                    nwv[:, :, None].to_broadcast((128, NCH, H)),
                )
                if debug and t == 0:
                    nc.sync.dma_start(dbg_rawn_d[:], rawn[:])

                em = wpool.tile([128, NCH, H], F16, tag="em")
                nc.scalar.activation(
                    em[:].rearrange("p c h -> p (c h)"),
                    rawn[:].rearrange("p c h -> p (c h)"), AFT.Exp,
                )

                # fused of tile t-2 (2-tile skew: the aggT copy has a full
                # iteration to land, so fused never stalls on it). Its LN is
                # issued later (after expblk) to keep the DVE queue clear
                # for the recS chain.
                ln_args = None
                if len(pending) == 2:
                    ln_args = issue_fused(*pending.pop(0))

                s_ps = srm[0:8].rearrange("p c h -> p (c h)")
                nc.tensor.matmul(
                    s_ps, bm8, em[:].rearrange("p c h -> p (c h)"),
                    start=True, stop=True,
                )
                s_eps = wpool.tile([8, NCH * H], F32, tag="seps")
                nc.vector.tensor_scalar_add(s_eps[:], s_ps, S_EPS)
                recS = wpool.tile([8, NCH * H], F16, tag="recS")
                with nc.allow_low_precision(reason="recS feeds fp16 matmul"):
                    nc.vector.reciprocal(recS[:], s_eps[:])

                # z half-1 for tile t+1 fills the recS latency on PE;
                # its tanh goes early so h1 is ready for raw(t+1)
                h_next = None
                zp1 = zp2 = None
                if t + 1 < NBT:
                    h_next = wpool.tile([128, 2048], F16, tag="h")
                    zp1 = issue_z_half(t + 1, tiles[t + 1][0], h_next, 0)
                    issue_tanh(h_next, zp1, 0)

                rmap_ps = srm[:].rearrange("p c h -> p (c h)")
                nc.tensor.matmul(rmap_ps, bm8T, recS[:], start=True, stop=True)

                # z half-2 fills the p/expblk latency before agg
                if t + 1 < NBT:
                    zp2 = issue_z_half(t + 1, tiles[t + 1][0], h_next, 1)

                p_sb = wpool.tile([128, NCH, H], F16, tag="p")
                nc.vector.tensor_mul(p_sb[:], em[:], srm[:])
                if debug and t == 0:
                    nc.sync.dma_start(dbg_em_d[:], em[:])
                    nc.sync.dma_start(dbg_p_d[:], p_sb[:])
                expblk = wpool.tile([128, NCH, 8, H], F16, tag="expblk")
                for half in range(2):
                    hc = slice(half * (NCH // 2), (half + 1) * (NCH // 2))
                    nc.vector.tensor_mul(
                        expblk[:, hc],
                        p_sb[:, hc, None, :].to_broadcast((128, NCH // 2, 8, H)),
                        bm16[:, None, :, :].to_broadcast((128, NCH // 2, 8, H)),
                    )
                if ln_args is not None:
                    issue_ln(*ln_args)

                # ---- aggT[dd, dh, (b,h)] += ner8_c.T @ expblk_c ----
                agg_ps = aggps_p.tile([128, 2, 2 * 128], F32, tag="aggT")
                for c in range(NCH):
                    for dh in range(2):
                        nc.tensor.matmul(
                            agg_ps[:, dh, 16 * c:16 * c + 16],
                            ner[:, c, dh * 128:(dh + 1) * 128],
                            expblk[:, c],
                            start=True, stop=True,
                        )
                aggT = wpool.tile([128, 2, 2 * 128], F16, tag="aggTsb", bufs=3)
                nc.scalar.copy(aggT[:], agg_ps[:])
                if debug and t == 0:
                    nc.sync.dma_start(dbg_aggT_d[:], aggT[:])
                # tanh of z half-2 (t+1) issued after the aggT copy so the
                # copy isn't stuck behind it in the ACT queue
                if zp2 is not None:
                    issue_tanh(h_next, zp2, 1)

                tiles.pop(t - 2, None)
                pending.append((t, aggT, cen))
                h_cur = h_next

            for args in pending:
                issue_ln(*issue_fused(*args))

    nc.finalize()
    return nc


def _f8(x):
    import ml_dtypes
    return np.clip(x, -240.0, 240.0).astype(ml_dtypes.float8_e4m3)


def _patch_ldw_opt():
    import concourse.bass_utils as _bu
    if getattr(_bu, "_ldwopt_patched", False):
        return
    _bu._ldwopt_patched = True


def kernel(center_emb, neighbor_embs, neighbor_weights, neighbor_valid,
           W1, b1, w2, Wc, bc, alpha, gamma, beta):
    _patch_ldw_opt()
    from concourse.bass_utils import run_bass_kernel_spmd

    global LAST_EXEC_NS

    f32 = np.float32
    f16 = np.float16
    ce = np.asarray(center_emb, f32)
    ne = np.asarray(neighbor_embs, f32)
    nw = np.asarray(neighbor_weights, f32)
    va = np.asarray(neighbor_valid)
    W1 = np.asarray(W1, f32)
    b1 = np.asarray(b1, f32)
    w2 = np.asarray(w2, f32)
    Wc = np.asarray(Wc, f32)
    bc = np.asarray(bc, f32)
    alpha = np.asarray(alpha, f32)
    gamma = np.asarray(gamma, f32)
    beta = np.asarray(beta, f32)

    apply_gamma_beta = not (np.all(gamma == 1.0) and np.all(beta == 0.0))
    apply_b1 = bool(np.any(b1 != 0.0))

    key = (apply_gamma_beta, apply_b1, bool(os.environ.get("NE_DEBUG_DUMP")))
    if key not in _prog_cache:
        _prog_cache[key] = _build_program(key[0], key[1])
    nc = _prog_cache[key]

    # ---- host-side const prep (weight folding + dtype casts + layouts) ----
    import ml_dtypes
    F8NP = ml_dtypes.float8_e4m3
    sig = 1.0 / (1.0 + np.exp(-float(alpha[0])))
    VU = np.concatenate([W1[h, D:2 * D] - W1[h, 2 * D:3 * D] for h in range(H)], axis=1)
    U2 = np.concatenate([W1[h, :D] + W1[h, 2 * D:3 * D] for h in range(H)], axis=1)
    # d = p + 128*i  ->  [p, i, cols];  vu | u2 side by side
    vu2 = np.concatenate([
        _f8(VU).reshape(2, 128, HA).transpose(1, 0, 2),
        _f8(U2).reshape(2, 128, HA).transpose(1, 0, 2),
    ], axis=2)
    vu2 = np.ascontiguousarray(vu2)

    cb = np.zeros((128, CBLOB), f16)
    for h in range(H):
        cb[h * A:(h + 1) * A, h] = w2[h].astype(f16)          # w2p [*,0:2]
    pidx = np.arange(128)
    cb[:, 2:10] = (pidx[:, None] // 16 == np.arange(8)[None, :])   # bm8
    for p in range(128):
        cb[p, 10 + (p // 16) * H:10 + (p // 16) * H + H] = 1.0     # bm16
    wcc_f = (Wc * sig).astype(f16).reshape(H, 2, 128, D)
    for h in range(H):
        for dh in range(2):
            c0 = 26 + (h * 2 + dh) * D
            cb[:, c0:c0 + D] = wcc_f[h, dh]
    cb[0:8, 1050:1178] = (pidx[None, :] // 16 == np.arange(8)[:, None])  # bm8T
    for pl in range(32):
        cb[pl, 1178 + pl * 16:1178 + (pl + 1) * 16] = 1.0       # idk
    cb[0, 1690:1818] = 1.0                                       # ones_row
    cb[:, 1818] = b1.reshape(HA).astype(f16)                     # b1 column

    gamma_r = gamma.reshape(1, D).astype(f32)
    beta_r = beta.reshape(1, D).astype(f32)

    nwv = np.where(va, nw, NWV_NEG).astype(f16)        # [B, K]
    ce_bc = (ce + (bc * sig)[None, :]).astype(f16)     # bc folded into center

    in_maps = []
    for cidx in range(NCORES):
        rs = slice(cidx * BC, (cidx + 1) * BC)
        ne_c = _f8(ne[rs].reshape(BC * K, D))          # [BC*K, D] fp8
        blob = np.zeros((NBT, 128, TBLOB), np.uint8)
        # neT8 [t, p, i, col]: ne[row(t,col), p+128i]
        neT8 = np.ascontiguousarray(
            ne_c.reshape(NBT, 2048, 2, 128).transpose(0, 3, 2, 1)
        )
        blob[:, :, 0:4096] = neT8.reshape(NBT, 128, 4096).view(np.uint8)
        # ner8 [t, p, c, d]: ne[t*2048 + c*128 + p, d]
        ner8 = np.ascontiguousarray(
            ne_c.reshape(NBT, NCH, 128, D).transpose(0, 2, 1, 3)
        )
        blob[:, :, 4096:8192] = ner8.reshape(NBT, 128, 4096).view(np.uint8)
        blob[:, :, 8192:8704] = np.ascontiguousarray(
            ce_bc[rs].reshape(NBT, 128, D)).view(np.uint8).reshape(NBT, 128, 512)
        blob[:, :, 8704:8736] = np.ascontiguousarray(
            nwv[rs].reshape(NBT, NCH, 128).transpose(0, 2, 1)
        ).view(np.uint8).reshape(NBT, 128, 32)
        ceT8 = np.ascontiguousarray(
            _f8(ce[rs]).reshape(BC, 2, 128).transpose(2, 1, 0)
        )
        in_maps.append({
            "blob8": blob.view(F8NP),
            "ceT8": ceT8,
            "vu2_8": vu2,
            "cblob16": cb,
            "gamma_r": gamma_r,
            "beta_r": beta_r,
        })

    trace = bool(os.environ.get("NE_KERNEL_TRACE"))
    if trace:
        _maybe_install_profile_hook()
    res = run_bass_kernel_spmd(nc, in_maps, list(range(NCORES)), trace=trace)
    LAST_EXEC_NS = res.exec_time_ns
    if trace:
        print("kernel exec_time_ns:", res.exec_time_ns, "mean:", res.mean_exec_time_ns)

    out = np.empty((B, D), f32)
    for cidx in range(NCORES):
        out[cidx * BC:(cidx + 1) * BC] = res.results[cidx]["out"]
    return out


# revision 26
# speedup vs baseline: 1.0499x; 1.0499x over previous
"""Trainium2 Bass kernel for nn_NeighborhoodAttentionModule.

Pure data-parallel over batch: B=16384 rows split as 2048 rows/core across 8
NeuronCores, 16 b-tiles of 128 rows per core. Per b-tile:

  s1T[b,(h,a)]   = ceT8.T @ U2         (PE DoubleRow fp8, 1 matmul)
  z[(h,a),(b,k)] = VU.T @ neT8 + s1T-broadcast   (PE: fp8 DoubleRow + fp16
                   rank-expand matmul accumulated into same PSUM)
  h = tanh(z)                          (ACT, PSUM->SBUF fp16)
  raw[(b,k),(c,h)] = h_chunk.T @ w2    (PE chunk-stationary, 16 tiny matmuls)
  rawn = raw + nwv                     (DVE; nwv = valid ? nw : -30, host-folded)
  em = exp(rawn)                       (ACT fp16; invalid -> exp(-30+raw) == 0)
  S[(b',h),(c,h)] via bm8 matmul; recS = 1/(S+2e-5)  (PE + DVE)
  recSmap = bm8T @ recS                (PE partition-broadcast)
  p = em * recSmap; expblk = p * blockmask           (DVE, fp16)
  aggT[dd,(b,h)] += ner8_chunk.T @ expblk_chunk      (PE, fp8 x fp16 mixed)
  fused = aggT.T @ Wcc + bc            (PE)
  out = LayerNorm(fused + center)      (DVE only: STT-accum stats + int
                                        rsqrt bithack + 2 Newton steps)

Zero ACT table swaps (Tanh+Exp share the exp_and_others table). All DRAM
tensors are laid out host-side as per-tile SBUF images (4KB/512B contiguous
partition lines) for dense DMA descriptors.
"""
import os
import numpy as np

B, K, D, H, A = 16384, 16, 256, 2, 64
TBLOB = 8736          # per-tile input blob bytes per partition
CBLOB = 1946          # fp16 const blob columns
NCORES = 8
BC = B // NCORES      # rows per core (2048)
NBT = BC // 128       # b-tiles per core (16)
NCH = 16              # chunks of 128 (b,k)-rows per b-tile
HA = H * A            # 128
EPS = 1e-5
NWV_NEG = -30.0       # folded invalid-neighbor bias
S_EPS = 2e-5          # S regularizer (valid rows always have S >= 1.5e-3)
RSQRT_MAGIC = 0x5F3759DF

LAST_EXEC_NS = None

_prog_cache = {}


def _maybe_install_profile_hook():
    """Optional NTFF profiling hook (for local testing only; fails soft)."""
    import sys, types, contextlib, ctypes

    if "antenv.axon_hooks" in sys.modules:
        return
    try:
        mod = types.ModuleType("antenv.axon_hooks")
        _state = {"hook": None}
        mod.set_axon_ntff_profile_hook = lambda h: _state.__setitem__("hook", h)
        mod.get_axon_ntff_profile_hook = lambda: _state["hook"]
        sys.modules["antenv.axon_hooks"] = mod
        import antenv

        antenv.axon_hooks = mod
        so_path = "/opt/axon/libaxon_pjrt.so"
        lib = ctypes.CDLL(so_path)
        if not hasattr(lib, "axon_start_nrt_profile"):
            return
        lib.axon_start_nrt_profile.argtypes = [
            ctypes.POINTER(ctypes.c_int64),
            ctypes.c_size_t,
        ]
        lib.axon_start_nrt_profile.restype = ctypes.c_int64
        lib.axon_stop_nrt_profile.argtypes = [ctypes.c_char_p]
        lib.axon_stop_nrt_profile.restype = ctypes.c_int64

        @contextlib.contextmanager
        def _hook(output_dir, device_ids):
            import jax

            jax.devices()
            if device_ids:
                ids = (ctypes.c_int64 * len(device_ids))(*device_ids)
                rc = lib.axon_start_nrt_profile(ids, len(device_ids))
            else:
                rc = lib.axon_start_nrt_profile(None, 0)
            if rc != 0:
                raise RuntimeError(f"axon_start_nrt_profile rc={rc}")
            try:
                yield
            finally:
                n = lib.axon_stop_nrt_profile(str(output_dir).encode())
                print(f"profile: {n} ntff file(s) -> {output_dir}")

        mod.set_axon_ntff_profile_hook(_hook)
    except Exception as e:  # noqa: BLE001
        print("profile hook unavailable:", e)


def _build_program(apply_gamma_beta: bool, apply_b1: bool):
    from concourse import bacc, tile, mybir

    F8 = mybir.dt.float8e4
    F16 = mybir.dt.float16
    F32 = mybir.dt.float32
    I32 = mybir.dt.int32
    AFT = mybir.ActivationFunctionType
    ALU = mybir.AluOpType
    PM = mybir.MatmulPerfMode

    nc = bacc.Bacc(None, target_bir_lowering=False)

    # ---- DRAM parameters (per-core shard; all per-tile SBUF images) ----
    dp = nc.declare_dram_parameter
    # per-tile blob: neT8 4096B | ner8 4096B | ce16 512B | nwv16 32B
    blob_d = dp("blob8", [NBT, 128, TBLOB], F8, isOutput=False)
    ceT_d = dp("ceT8", [128, 2, BC], F8, isOutput=False)
    vu2_d = dp("vu2_8", [128, 2, 2 * HA], F8, isOutput=False)  # vu | u2
    # fp16 const blob: w2p 2 | bm8 8 | bm16 16 | wcc 1024 | bm8T 128 |
    # idk 512 | ones 128 | b1 128
    cb_d = dp("cblob16", [128, CBLOB], F16, isOutput=False)
    gam_d = dp("gamma_r", [1, D], F32, isOutput=False)
    bet_d = dp("beta_r", [1, D], F32, isOutput=False)
    out_d = dp("out", [BC, D], F32, isOutput=True)
    debug = bool(os.environ.get("NE_DEBUG_DUMP"))
    if debug:
        dbg_h_d = dp("dbg_h", [128, 2048], F16, isOutput=True)
        dbg_rawn_d = dp("dbg_rawn", [128, NCH, H], F32, isOutput=True)
        dbg_em_d = dp("dbg_em", [128, NCH, H], F16, isOutput=True)
        dbg_p_d = dp("dbg_p", [128, NCH, H], F16, isOutput=True)
        dbg_aggT_d = dp("dbg_aggT", [128, 2, 2 * 128], F16, isOutput=True)
        dbg_x_d = dp("dbg_x", [128, D], F32, isOutput=True)

    with tile.TileContext(nc) as tc:
        with (
            tc.tile_pool(name="const", bufs=1) as cpool,
            tc.tile_pool(name="loads", bufs=5) as lpool,
            tc.tile_pool(name="work", bufs=2) as wpool,
            tc.tile_pool(name="zps", bufs=2, space="PSUM") as zps_p,
            tc.tile_pool(name="srm_ps", bufs=2, space="PSUM") as srm_p,
            tc.tile_pool(name="aggps", bufs=1, space="PSUM") as aggps_p,
            tc.tile_pool(name="fups", bufs=1, space="PSUM") as fups_p,
        ):
            def cload(name, dram_ap, shape, dt):
                t = cpool.tile(shape, dt, tag=name, name=name)
                nc.sync.dma_start(t[:], dram_ap)
                return t

            ceT8 = cload("ceT8", ceT_d[:], [128, 2, BC], F8)
            vu2 = cload("vu2", vu2_d[:], [128, 2, 2 * HA], F8)
            vu8 = vu2[:, :, 0:HA]
            u28 = vu2[:, :, HA:2 * HA]
            cb = cload("cb", cb_d[:], [128, CBLOB], F16)
            w2p = cb[:, 0:2]
            bm8 = cb[:, 2:10]
            bm16 = cb[:, 10:26].rearrange("p (b h) -> p b h", h=H)
            wcc = [[cb[:, 26 + (h * 2 + dh) * D:26 + (h * 2 + dh + 1) * D]
                    for dh in range(2)] for h in range(2)]
            bm8T = cb[0:8, 1050:1178]
            if apply_b1:
                b1c = cpool.tile([128, 1], F32, tag="b1c")
                nc.vector.tensor_copy(b1c[:], cb[:, 1818:1819])
            gam_t = (
                cload("gam", gam_d[:].to_broadcast((128, D)), [128, D], F32)
                if apply_gamma_beta else None
            )
            bet_t = (
                cload("bet", bet_d[:].to_broadcast((128, D)), [128, D], F32)
                if apply_gamma_beta else None
            )

            def issue_loads(t):
                blob = lpool.tile([128, TBLOB], F8, tag="blob")
                nc.sync.dma_start(blob[:], blob_d[t])
                neT = blob[:, 0:4096].rearrange("p (i c) -> p i c", i=2)
                ner = blob[:, 4096:8192].rearrange("p (c d) -> p c d", c=NCH)
                cen = blob[:, 8192:8704].bitcast(F16)
                nwv = blob[:, 8704:8736].bitcast(F16)
                return neT, ner, cen, nwv

            def issue_fused(t, aggT, cen):
                # fused = combined @ Wc (bc folded into cen host-side)
                fu_ps = fups_p.tile([128, D], F32, tag="fu")
                mms = [(h, dh) for h in range(2) for dh in range(2)]
                for i, (h, dh) in enumerate(mms):
                    lhs = aggT[:, dh].rearrange("p (b h) -> p h b", h=2)[:, h, :]
                    nc.tensor.matmul(
                        fu_ps[:], lhs, wcc[h][dh],
                        start=(i == 0), stop=(i == 3),
                    )
                return (t, fu_ps, cen)

            def issue_ln(t, fu_ps, cen):
                # residual + layernorm (all DVE)
                x_t = wpool.tile([128, D], F32, tag="x")
                msum = wpool.tile([128, 1], F32, tag="msum")
                nc.vector.scalar_tensor_tensor(
                    x_t[:], fu_ps[:], 1.0, cen[:],
                    op0=ALU.mult, op1=ALU.add, accum_out=msum[:],
                )
                if debug and t == 0:
                    nc.sync.dma_start(dbg_x_d[:], x_t[:])
                negmean = wpool.tile([128, 1], F32, tag="negmean")
                nc.vector.tensor_scalar_mul(negmean[:], msum[:], -1.0 / D)
                sq_t = wpool.tile([128, D], F32, tag="sq")
                sumsq = wpool.tile([128, 1], F32, tag="sumsq")
                nc.vector.scalar_tensor_tensor(
                    sq_t[:], x_t[:], 1.0, x_t[:],
                    op0=ALU.mult, op1=ALU.mult, accum_out=sumsq[:],
                )
                m2 = wpool.tile([128, 1], F32, tag="m2")
                nc.vector.tensor_mul(m2[:], negmean[:], negmean[:])
                q_t = wpool.tile([128, 1], F32, tag="q")
                nc.vector.tensor_scalar(
                    q_t[:], sumsq[:], 1.0 / D, EPS, op0=ALU.mult, op1=ALU.add,
                )
                nc.vector.tensor_sub(q_t[:], q_t[:], m2[:])
                # invstd = rsqrt(q): int bithack + 2 Newton steps
                yi = wpool.tile([128, 1], I32, tag="yi")
                nc.vector.tensor_scalar(
                    yi[:], q_t[:].bitcast(I32), 1, None,
                    op0=ALU.logical_shift_right,
                )
                nc.vector.tensor_scalar(
                    yi[:], yi[:], RSQRT_MAGIC, -1,
                    op0=ALU.subtract, op1=ALU.mult,
                )
                y = yi[:].bitcast(F32)
                nr1 = wpool.tile([128, 1], F32, tag="nr1")
                nr2 = wpool.tile([128, 1], F32, tag="nr2")
                for _ in range(2):
                    nc.vector.tensor_mul(nr1[:], y, y)
                    nc.vector.scalar_tensor_tensor(
                        nr2[:], q_t[:], -0.5, nr1[:], op0=ALU.mult, op1=ALU.mult,
                    )
                    nc.vector.tensor_scalar(nr1[:], nr2[:], 1.5, None, op0=ALU.add)
                    nc.vector.tensor_mul(yi[:].bitcast(F32), y, nr1[:])
                xn = wpool.tile([128, D], F32, tag="xn")
                nc.vector.tensor_scalar(
                    xn[:], x_t[:], negmean[:], yi[:].bitcast(F32),
                    op0=ALU.add, op1=ALU.mult,
                )
                if apply_gamma_beta:
                    nc.vector.tensor_mul(xn[:], xn[:], gam_t[:])
                    nc.vector.tensor_add(xn[:], xn[:], bet_t[:])
                nc.sync.dma_start(out_d[t * 128:(t + 1) * 128, :], xn[:])

            def issue_z_half(t, neT, h_sb, hf):
                # z = VU.T @ neT8 + U2.T @ ceT8-kexp (all fp8 DoubleRow; the
                # ce pass uses a stride-0 k-broadcast moving AP).
                z_ps = zps_p.tile([128, 1024], F32, tag="z")
                for m in range(4):
                    c0 = hf * 1024 + m * 256
                    b0 = t * 128 + hf * 64 + m * 16
                    zq = z_ps[:, m * 256:(m + 1) * 256]
                    nc.tensor.matmul(
                        zq, vu8, neT[:, :, c0:c0 + 256],
                        start=True, stop=False, perf_mode=PM.DoubleRow,
                        skip_group_check=True,
                    )
                    ce_kexp = ceT8[:, :, b0:b0 + 16][:, :, :, None] \
                        .to_broadcast((128, 2, 16, 16))
                    nc.tensor.matmul(
                        zq, u28, ce_kexp,
                        start=False, stop=True, perf_mode=PM.DoubleRow,
                        skip_group_check=True,
                    )
                return z_ps

            def issue_tanh(h_sb, z_ps, hf):
                if apply_b1:
                    nc.scalar.activation(
                        h_sb[:, hf * 1024:(hf + 1) * 1024], z_ps[:],
                        AFT.Tanh, bias=b1c[:],
                    )
                else:
                    nc.scalar.activation(
                        h_sb[:, hf * 1024:(hf + 1) * 1024], z_ps[:], AFT.Tanh,
                    )

            # ---- software-pipelined main loop ----
            # iteration t: raw/em/S/agg for tile t, fused+LN for t-1,
            # z+tanh for t+1, blob prefetch for t+2.
            tiles = {0: issue_loads(0)}
            tiles[1] = issue_loads(1)
            h_cur = wpool.tile([128, 2048], F16, tag="h")
            for hf in range(2):
                zp = issue_z_half(0, tiles[0][0], h_cur, hf)
                issue_tanh(h_cur, zp, hf)
            pending = []
            for t in range(NBT):
                neT, ner, cen, nwv = tiles[t]
                if t + 2 < NBT:
                    tiles[t + 2] = issue_loads(t + 2)
                if debug and t == 0:
                    nc.sync.dma_start(dbg_h_d[:], h_cur[:])

                # ---- raw scores (chunk-stationary) + nwv add ----
                srm = srm_p.tile([128, NCH, H], F32, tag="srm")
                raw_ps = srm
                for c in range(NCH):
                    nc.tensor.matmul(
                        raw_ps[:, c, :],
                        h_cur[:, c * 128:(c + 1) * 128], w2p,
                        start=True, stop=True,
                    )
                rawn = wpool.tile([128, NCH, H], F32, tag="rawn")
                nc.vector.tensor_add(
                    rawn[:], raw_ps[:],
                    nwv[:, :, None].to_broadcast((128, NCH, H)),
                )
                if debug and t == 0:
                    nc.sync.dma_start(dbg_rawn_d[:], rawn[:])

                em = wpool.tile([128, NCH, H], F16, tag="em")
                nc.scalar.activation(
                    em[:].rearrange("p c h -> p (c h)"),
                    rawn[:].rearrange("p c h -> p (c h)"), AFT.Exp,
                )

                # fused of tile t-2 (2-tile skew: the aggT copy has a full
                # iteration to land, so fused never stalls on it). Its LN is
                # issued later (after expblk) to keep the DVE queue clear
                # for the recS chain.
                ln_args = None
                if len(pending) == 2:
                    ln_args = issue_fused(*pending.pop(0))

                s_ps = srm[0:8].rearrange("p c h -> p (c h)")
                nc.tensor.matmul(
                    s_ps, bm8, em[:].rearrange("p c h -> p (c h)"),
                    start=True, stop=True,
                )
                s_eps = wpool.tile([8, NCH * H], F32, tag="seps")
                nc.vector.tensor_scalar_add(s_eps[:], s_ps, S_EPS)
                recS = wpool.tile([8, NCH * H], F16, tag="recS")
                with nc.allow_low_precision(reason="recS feeds fp16 matmul"):
                    nc.vector.reciprocal(recS[:], s_eps[:])

                # z half-1 for tile t+1 fills the recS latency on PE;
                # its tanh goes early so h1 is ready for raw(t+1)
                h_next = None
                zp1 = zp2 = None
                if t + 1 < NBT:
                    h_next = wpool.tile([128, 2048], F16, tag="h")
                    zp1 = issue_z_half(t + 1, tiles[t + 1][0], h_next, 0)
                    issue_tanh(h_next, zp1, 0)

                rmap_ps = srm[:].rearrange("p c h -> p (c h)")
                nc.tensor.matmul(rmap_ps, bm8T, recS[:], start=True, stop=True)

                # z half-2 fills the p/expblk latency before agg
                if t + 1 < NBT:
                    zp2 = issue_z_half(t + 1, tiles[t + 1][0], h_next, 1)

                p_sb = wpool.tile([128, NCH, H], F16, tag="p")
                nc.vector.tensor_mul(p_sb[:], em[:], srm[:])
                if debug and t == 0:
                    nc.sync.dma_start(dbg_em_d[:], em[:])
                    nc.sync.dma_start(dbg_p_d[:], p_sb[:])
                expblk = wpool.tile([128, NCH, 8, H], F16, tag="expblk")
                for half in range(2):
                    hc = slice(half * (NCH // 2), (half + 1) * (NCH // 2))
                    nc.vector.tensor_mul(
                        expblk[:, hc],
                        p_sb[:, hc, None, :].to_broadcast((128, NCH // 2, 8, H)),
                        bm16[:, None, :, :].to_broadcast((128, NCH // 2, 8, H)),
                    )
                if ln_args is not None:
                    issue_ln(*ln_args)

                # ---- aggT[dd, dh, (b,h)] += ner8_c.T @ expblk_c ----
                agg_ps = aggps_p.tile([128, 2, 2 * 128], F32, tag="aggT")
                for c in range(NCH):
                    for dh in range(2):
                        nc.tensor.matmul(
                            agg_ps[:, dh, 16 * c:16 * c + 16],
                            ner[:, c, dh * 128:(dh + 1) * 128],
                            expblk[:, c],
                            start=True, stop=True,
                        )
                aggT = wpool.tile([128, 2, 2 * 128], F16, tag="aggTsb", bufs=3)
                nc.scalar.copy(aggT[:], agg_ps[:])
                if debug and t == 0:
                    nc.sync.dma_start(dbg_aggT_d[:], aggT[:])
                # tanh of z half-2 (t+1) issued after the aggT copy so the
                # copy isn't stuck behind it in the ACT queue
                if zp2 is not None:
                    issue_tanh(h_next, zp2, 1)

                tiles.pop(t - 2, None)
                pending.append((t, aggT, cen))
                h_cur = h_next

            for args in pending:
                issue_ln(*issue_fused(*args))

    nc.finalize()
    return nc


def _f8(x):
    import ml_dtypes
    return np.clip(x, -240.0, 240.0).astype(ml_dtypes.float8_e4m3)


def _patch_ldw_opt():
    import concourse.bass_utils as _bu
    if getattr(_bu, "_ldwopt_patched", False):
        return
    _bu._ldwopt_patched = True


def kernel(center_emb, neighbor_embs, neighbor_weights, neighbor_valid,
           W1, b1, w2, Wc, bc, alpha, gamma, beta):
    _patch_ldw_opt()
    from concourse.bass_utils import run_bass_kernel_spmd

    global LAST_EXEC_NS

    f32 = np.float32
    f16 = np.float16
    ce = np.asarray(center_emb, f32)
    ne = np.asarray(neighbor_embs, f32)
    nw = np.asarray(neighbor_weights, f32)
    va = np.asarray(neighbor_valid)
    W1 = np.asarray(W1, f32)
    b1 = np.asarray(b1, f32)
    w2 = np.asarray(w2, f32)
    Wc = np.asarray(Wc, f32)
    bc = np.asarray(bc, f32)
    alpha = np.asarray(alpha, f32)
    gamma = np.asarray(gamma, f32)
    beta = np.asarray(beta, f32)

    apply_gamma_beta = not (np.all(gamma == 1.0) and np.all(beta == 0.0))
    apply_b1 = bool(np.any(b1 != 0.0))

    key = (apply_gamma_beta, apply_b1, bool(os.environ.get("NE_DEBUG_DUMP")))
    if key not in _prog_cache:
        _prog_cache[key] = _build_program(key[0], key[1])
    nc = _prog_cache[key]

    # ---- host-side const prep (weight folding + dtype casts + layouts) ----
    import ml_dtypes
    F8NP = ml_dtypes.float8_e4m3
    sig = 1.0 / (1.0 + np.exp(-float(alpha[0])))
    VU = np.concatenate([W1[h, D:2 * D] - W1[h, 2 * D:3 * D] for h in range(H)], axis=1)
    U2 = np.concatenate([W1[h, :D] + W1[h, 2 * D:3 * D] for h in range(H)], axis=1)
    # d = p + 128*i  ->  [p, i, cols];  vu | u2 side by side
    vu2 = np.concatenate([
        _f8(VU).reshape(2, 128, HA).transpose(1, 0, 2),
        _f8(U2).reshape(2, 128, HA).transpose(1, 0, 2),
    ], axis=2)
    vu2 = np.ascontiguousarray(vu2)

    cb = np.zeros((128, CBLOB), f16)
    for h in range(H):
        cb[h * A:(h + 1) * A, h] = w2[h].astype(f16)          # w2p [*,0:2]
    pidx = np.arange(128)
    cb[:, 2:10] = (pidx[:, None] // 16 == np.arange(8)[None, :])   # bm8
    for p in range(128):
        cb[p, 10 + (p // 16) * H:10 + (p // 16) * H + H] = 1.0     # bm16
    wcc_f = (Wc * sig).astype(f16).reshape(H, 2, 128, D)
    for h in range(H):
        for dh in range(2):
            c0 = 26 + (h * 2 + dh) * D
            cb[:, c0:c0 + D] = wcc_f[h, dh]
    cb[0:8, 1050:1178] = (pidx[None, :] // 16 == np.arange(8)[:, None])  # bm8T
    for pl in range(32):
        cb[pl, 1178 + pl * 16:1178 + (pl + 1) * 16] = 1.0       # idk
    cb[0, 1690:1818] = 1.0                                       # ones_row
    cb[:, 1818] = b1.reshape(HA).astype(f16)                     # b1 column

    gamma_r = gamma.reshape(1, D).astype(f32)
    beta_r = beta.reshape(1, D).astype(f32)

    nwv = np.where(va, nw, NWV_NEG).astype(f16)        # [B, K]
    ce_bc = (ce + (bc * sig)[None, :]).astype(f16)     # bc folded into center

    in_maps = []
    for cidx in range(NCORES):
        rs = slice(cidx * BC, (cidx + 1) * BC)
        ne_c = _f8(ne[rs].reshape(BC * K, D))          # [BC*K, D] fp8
        blob = np.zeros((NBT, 128, TBLOB), np.uint8)
        # neT8 [t, p, i, col]: ne[row(t,col), p+128i]
        neT8 = np.ascontiguousarray(
            ne_c.reshape(NBT, 2048, 2, 128).transpose(0, 3, 2, 1)
        )
        blob[:, :, 0:4096] = neT8.reshape(NBT, 128, 4096).view(np.uint8)
        # ner8 [t, p, c, d]: ne[t*2048 + c*128 + p, d]
        ner8 = np.ascontiguousarray(
            ne_c.reshape(NBT, NCH, 128, D).transpose(0, 2, 1, 3)
        )
        blob[:, :, 4096:8192] = ner8.reshape(NBT, 128, 4096).view(np.uint8)
        blob[:, :, 8192:8704] = np.ascontiguousarray(
            ce_bc[rs].reshape(NBT, 128, D)).view(np.uint8).reshape(NBT, 128, 512)
        blob[:, :, 8704:8736] = np.ascontiguousarray(
            nwv[rs].reshape(NBT, NCH, 128).transpose(0, 2, 1)
        ).view(np.uint8).reshape(NBT, 128, 32)
        ceT8 = np.ascontiguousarray(
            _f8(ce[rs]).reshape(BC, 2, 128).transpose(2, 1, 0)
        )
        in_maps.append({
            "blob8": blob.view(F8NP),
            "ceT8": ceT8,
            "vu2_8": vu2,
            "cblob16": cb,
            "gamma_r": gamma_r,
            "beta_r": beta_r,
        })

    trace = bool(os.environ.get("NE_KERNEL_TRACE"))
    if trace:
        _maybe_install_profile_hook()
    res = run_bass_kernel_spmd(nc, in_maps, list(range(NCORES)), trace=trace)
    LAST_EXEC_NS = res.exec_time_ns
    if trace:
        print("kernel exec_time_ns:", res.exec_time_ns, "mean:", res.mean_exec_time_ns)

    out = np.empty((B, D), f32)
    for cidx in range(NCORES):
        out[cidx * BC:(cidx + 1) * BC] = res.results[cidx]["out"]
    return out
